# revision 8
# baseline (speedup 1.0000x reference)
"""DTCWT forward (3-level) Trainium2 Bass kernel.

Self-contained: builds banded-operator constant matrices from the filter
taps, shards the 8x16 (batch,channel) images across 8 NeuronCores (16
images each), and runs the whole transform as fp32r TensorEngine matmuls.

Factorization per 256x256 image (validated offline to 1e-16 vs reference):
  L1 stage1:  psV = xT-blocks @ [Fe.T|Fo.T]      (x as stationary lhsT)
  L1 stage2:  band-pair tiles  [T+|T-] = V_a@[A|A] + V_b@[B|-B]  in PSUM,
              producing q2c-combined re/im-interleaved outputs directly.
  ll path:    [ll_e|ll_o] = V3@G0llT , V4@G0llT  (row-parity split rows)
  L2/L3: same structure with dual-tree decimated operators; level inputs
  stay in row-parity-split layout, folded into the next level's constants.
"""

import numpy as np

import concourse.bacc as bacc
import concourse.mybir as mybir
from concourse import tile
from concourse.bass_utils import run_bass_kernel_spmd
from neuron_dtypes import static_cast_fp32_to_fp32r

F32 = mybir.dt.float32
F32R = mybir.dt.float32r
SQ2 = float(1.0 / np.sqrt(2.0))

N_CORES = 8
N_IMG = 16  # images per core


# ----------------------------------------------------------------------------
# Host-side constant operator matrices (float64)
# ----------------------------------------------------------------------------

def _corr(Xp, hr, Lout, stride):
    m = hr.shape[0]
    hi = stride * (Lout - 1) + 1
    acc = hr[0] * Xp[0:hi:stride]
    for k in range(1, m):
        acc = acc + hr[k] * Xp[k:k + hi:stride]
    return acc


def _colfilter_mat(h, L):
    m = h.shape[0]
    m2 = m // 2
    X = np.eye(L, dtype=np.float64)
    Xp = np.pad(X, ((m2, m2), (0, 0)), mode='symmetric')
    return _corr(Xp, h[::-1].astype(np.float64), L, 1)


def _coldfilt_mat(ha, hb, highpass, L):
    X = np.eye(L, dtype=np.float64)
    Xp = np.pad(X, ((ha.shape[0], ha.shape[0]), (0, 0)), mode='symmetric')
    E = Xp[2::2]
    O = Xp[3::2]
    a_out = _corr(O, ha[::-1].astype(np.float64), L // 4, 2)
    b_out = _corr(E, hb[::-1].astype(np.float64), L // 4, 2)
    ev, od = (b_out, a_out) if highpass else (a_out, b_out)
    out = np.zeros((L // 2, L), dtype=np.float64)
    out[0::2] = ev
    out[1::2] = od
    return out


def _pair_consts(R, n_j):
    """A/B matrices [L, 2*n_j]: T+ = Va@A + Vb@B, T- = Va@A - Vb@B give the
    q2c column combines (a-d, b+c) / (a+d, b-c) re/im-interleaved."""
    Re = R[0::2]
    Ro = R[1::2]
    L = R.shape[1]
    A = np.zeros((L, 2 * n_j))
    B = np.zeros((L, 2 * n_j))
    A[:, 0::2] = Re.T
    A[:, 1::2] = Ro.T
    B[:, 0::2] = -Ro.T
    B[:, 1::2] = Re.T
    return A, B


def build_consts(h0o, h1o, h0a, h0b, h1a, h1b):
    C = {}
    # L1 (undecimated, 256)
    F0 = _colfilter_mat(h0o, 256)
    F1 = _colfilter_mat(h1o, 256)
    F1e, F1o = SQ2 * F1[0::2], SQ2 * F1[1::2]
    F0e, F0o = SQ2 * F0[0::2], SQ2 * F0[1::2]
    C['RH01'] = np.concatenate([F1e.T, F1o.T, F0e.T, F0o.T], axis=1)  # [256,512]
    A1, B1 = _pair_consts(F0, 128)
    A2, B2 = _pair_consts(F1, 128)
    C['P1a'] = np.concatenate([A1, A1], axis=1)                 # [256,512]
    C['P1b'] = np.concatenate([B1, -B1], axis=1)
    C['P2a'] = np.concatenate([A2, A2], axis=1)
    C['P2b'] = np.concatenate([B2, -B2], axis=1)
    C['G0ll'] = (1.0 / SQ2) * F0.T                              # [256,256]
    # L2 (256 -> 128)
    C0 = _coldfilt_mat(h0b, h0a, False, 256)
    C1 = _coldfilt_mat(h1b, h1a, True, 256)
    C1e, C1o = SQ2 * C1[0::2], SQ2 * C1[1::2]
    C0e, C0o = SQ2 * C0[0::2], SQ2 * C0[1::2]
    RH2 = np.concatenate([C1e.T, C1o.T, C0e.T, C0o.T], axis=1)  # [256,256]
    C['RH2e'] = RH2[0::2]                                       # [128,256]
    C['RH2o'] = RH2[1::2]
    A1_, B1_ = _pair_consts(C0, 64)
    A2_, B2_ = _pair_consts(C1, 64)
    C['L2g1a'] = np.concatenate([A1_, A2_, A2_, A1_], axis=1)   # [256,512]
    C['L2g1b'] = np.concatenate([B1_, B2_, -B2_, -B1_], axis=1)
    Z = np.zeros((256, 128))
    C['L2g2a'] = np.concatenate([A2_, A2_, (1 / SQ2) * C0.T, Z], axis=1)
    C['L2g2b'] = np.concatenate([B2_, -B2_, Z, (1 / SQ2) * C0.T], axis=1)
    # L3 (128 -> 64)
    C0_ = _coldfilt_mat(h0b, h0a, False, 128)
    C1_ = _coldfilt_mat(h1b, h1a, True, 128)
    C1pe, C1po = SQ2 * C1_[0::2], SQ2 * C1_[1::2]
    C0pe, C0po = SQ2 * C0_[0::2], SQ2 * C0_[1::2]
    RH3 = np.concatenate([C1pe.T, C1po.T, C0pe.T, C0po.T], axis=1)  # [128,128]
    C['RH3e'] = RH3[0::2]                                       # [64,128]
    C['RH3o'] = RH3[1::2]
    A1__, B1__ = _pair_consts(C0_, 32)
    A2__, B2__ = _pair_consts(C1_, 32)
    C['L3g1a'] = np.concatenate([A1__, A2__, A2__, A1__], axis=1)   # [128,256]
    C['L3g1b'] = np.concatenate([B1__, B2__, -B2__, -B1__], axis=1)
    Z3 = np.zeros((128, 64))
    C['L3g2a'] = np.concatenate([A2__, A2__, (1 / SQ2) * C0_.T, Z3], axis=1)
    C['L3g2b'] = np.concatenate([B2__, -B2__, Z3, (1 / SQ2) * C0_.T], axis=1)
    return C


CONST_SHAPES = {
    'RH01': (256, 512),
    'P1a': (256, 512), 'P1b': (256, 512), 'P2a': (256, 512), 'P2b': (256, 512),
    'G0ll': (256, 256),
    'RH2e': (128, 256), 'RH2o': (128, 256),
    'L2g1a': (256, 512), 'L2g1b': (256, 512),
    'L2g2a': (256, 512), 'L2g2b': (256, 512),
    'RH3e': (64, 128), 'RH3o': (64, 128),
    'L3g1a': (128, 256), 'L3g1b': (128, 256),
    'L3g2a': (128, 256), 'L3g2b': (128, 256),
}


def _round_f32r(a):
    a = np.ascontiguousarray(a.astype(np.float32))
    return static_cast_fp32_to_fp32r(a).view(np.float32).reshape(a.shape)


# ----------------------------------------------------------------------------
# Device program
# ----------------------------------------------------------------------------

def build_nc(n_img=N_IMG):
    nc = bacc.Bacc("TRN2", target_bir_lowering=False, debug=False)

    x = nc.dram_tensor("x", [n_img, 256, 256], F32R, kind="ExternalInput")
    d_ll = nc.dram_tensor("ll", [n_img, 64, 64], F32, kind="ExternalOutput")
    d_y0 = nc.dram_tensor("yh0", [n_img, 6, 128, 128, 2], F32, kind="ExternalOutput")
    d_y1 = nc.dram_tensor("yh1", [n_img, 6, 64, 64, 2], F32, kind="ExternalOutput")
    d_y2 = nc.dram_tensor("yh2", [n_img, 6, 32, 32, 2], F32, kind="ExternalOutput")

    d_c = {k: nc.dram_tensor("c_" + k, list(v), F32R, kind="ExternalInput")
           for k, v in CONST_SHAPES.items()}

    # output-staging group sizes (images per DMA)
    G0 = min(2, n_img)
    G1 = min(2, n_img)
    G2 = min(4, n_img)
    GL = min(8, n_img)
    GX = min(2, n_img)

    with tile.TileContext(nc) as tc:
        with tc.tile_pool(name="consts", bufs=1) as cpool, \
             tc.tile_pool(name="work", bufs=3) as wp, \
             tc.tile_pool(name="stage", bufs=2) as sp, \
             tc.tile_pool(name="psv", bufs=3, space="PSUM") as psv, \
             tc.tile_pool(name="pst", bufs=5, space="PSUM") as pst:

            st = {}           # per-image / per-group live tiles
            cp_i = [0]

            def cp(dst, src):
                # alternate copies between ScalarE and VectorE
                if cp_i[0] % 2 == 0:
                    nc.scalar.copy(dst, src)
                else:
                    nc.vector.tensor_copy(dst, src)
                cp_i[0] += 1

            # ---- constants on the ACT HWDGE ring (keeps SP free for x/outputs)
            c = {}
            for ci, (k, (K, N)) in enumerate(CONST_SHAPES.items()):
                eng = nc.scalar if ci % 2 == 0 else nc.sync
                if K > 128:
                    t = cpool.tile([128, K // 128, N], F32R, tag="c_" + k)
                    eng.dma_start(
                        t[:], d_c[k].ap().rearrange("(kc p) n -> p kc n", p=128))
                else:
                    t = cpool.tile([K, N], F32R, tag="c_" + k)
                    eng.dma_start(t[:], d_c[k].ap())
                c[k] = t

            def emit_L1(im):
                g, q = im // GX, im % GX
                if q == 0:
                    sbx = wp.tile([128, GX, 2, 256], F32R, tag="sbx")
                    nc.sync.dma_start(
                        sbx[:], x.ap()[g * GX:(g + 1) * GX].rearrange(
                            "im (hc p) w -> p im hc w", p=128))
                    st['x', g] = sbx
                sbx = st['x', g]
                psVa = psv.tile([128, 512], F32, tag="psv")
                psVb = psv.tile([128, 512], F32, tag="psv")
                for wb, psV in ((0, psVa), (1, psVb)):
                    for hc in range(2):
                        nc.tensor.matmul(psV[:], sbx[:, q, hc, wb * 128:(wb + 1) * 128],
                                         c['RH01'][:, hc, :],
                                         start=(hc == 0), stop=(hc == 1))
                sbV = wp.tile([128, 2, 512], F32R, tag="sbv")
                cp(sbV[:, 0, :], psVa[:])
                cp(sbV[:, 1, :], psVb[:])

                g0, q0 = im // G0, im % G0
                if q0 == 0:
                    sb_y0 = sp.tile([128, G0, 6, 256], F32, tag="sby0")
                    st['y0', g0] = sb_y0
                sb_y0 = st['y0', g0]
                # band pairs (bands b0,b1 per pair): psT = [T+ | T-]
                for base, ka, kb, b0, b1 in ((0, 'P1a', 'P1b', 0, 5),
                                             (0, 'P2a', 'P2b', 1, 4),
                                             (256, 'P2a', 'P2b', 2, 3)):
                    psT = pst.tile([128, 512], F32, tag="pst")
                    for c2 in range(2):
                        nc.tensor.matmul(psT[:], sbV[:, c2, base:base + 128],
                                         c[ka][:, c2, :], start=(c2 == 0), stop=False)
                        nc.tensor.matmul(psT[:], sbV[:, c2, base + 128:base + 256],
                                         c[kb][:, c2, :], start=False, stop=(c2 == 1))
                    cp(sb_y0[:, q0, b0, :], psT[:, 0:256])
                    cp(sb_y0[:, q0, b1, :], psT[:, 256:512])
                if q0 == G0 - 1 or im == n_img - 1:
                    lo = g0 * G0
                    nc.sync.dma_start(
                        d_y0.ap()[lo:im + 1].rearrange("im b i j r -> i im b (j r)"),
                        sb_y0[:, 0:im + 1 - lo, :, :])

                # ll pack [T_lle | T_llo]
                ps_ll = pst.tile([128, 512], F32, tag="pst")
                for c2 in range(2):
                    nc.tensor.matmul(ps_ll[:, 0:256], sbV[:, c2, 256:384],
                                     c['G0ll'][:, c2, :], start=(c2 == 0), stop=(c2 == 1))
                for c2 in range(2):
                    nc.tensor.matmul(ps_ll[:, 256:512], sbV[:, c2, 384:512],
                                     c['G0ll'][:, c2, :], start=(c2 == 0), stop=(c2 == 1))
                sb_ll = wp.tile([128, 512], F32R, tag="sbll")
                cp(sb_ll[:], ps_ll[:])
                st['ll', im] = sb_ll

            def emit_L2(im):
                sb_ll = st.pop(('ll', im))
                psW = psv.tile([128, 512], F32, tag="psv")
                for wb in range(2):
                    for eo in range(2):
                        lhsT = sb_ll[:, eo * 256 + wb * 128: eo * 256 + (wb + 1) * 128]
                        rhs = c['RH2e'] if eo == 0 else c['RH2o']
                        nc.tensor.matmul(psW[:, wb * 256:(wb + 1) * 256],
                                         lhsT, rhs[:], start=(eo == 0), stop=(eo == 1))
                sbW = wp.tile([128, 512], F32R, tag="sbw")
                cp(sbW[:], psW[:])

                psG1 = pst.tile([64, 512], F32, tag="pst")
                for c2 in range(2):
                    nc.tensor.matmul(psG1[:], sbW[:, c2 * 256 + 0:c2 * 256 + 64],
                                     c['L2g1a'][:, c2, :], start=(c2 == 0), stop=False)
                    nc.tensor.matmul(psG1[:], sbW[:, c2 * 256 + 64:c2 * 256 + 128],
                                     c['L2g1b'][:, c2, :], start=False, stop=(c2 == 1))
                psG2 = pst.tile([64, 512], F32, tag="pst")
                for c2 in range(2):
                    nc.tensor.matmul(psG2[:], sbW[:, c2 * 256 + 128:c2 * 256 + 192],
                                     c['L2g2a'][:, c2, :], start=(c2 == 0), stop=False)
                    nc.tensor.matmul(psG2[:], sbW[:, c2 * 256 + 192:c2 * 256 + 256],
                                     c['L2g2b'][:, c2, :], start=False, stop=(c2 == 1))

                g1, q1 = im // G1, im % G1
                if q1 == 0:
                    sb_y1 = sp.tile([64, G1, 6, 128], F32, tag="sby1")
                    st['y1', g1] = sb_y1
                sb_y1 = st['y1', g1]
                cp(sb_y1[:, q1, 0:2, :], psG1[:, 0:256])
                cp(sb_y1[:, q1, 4:6, :], psG1[:, 256:512])
                cp(sb_y1[:, q1, 2:4, :], psG2[:, 0:256])
                if q1 == G1 - 1 or im == n_img - 1:
                    lo = g1 * G1
                    nc.sync.dma_start(
                        d_y1.ap()[lo:im + 1].rearrange("im b i j r -> i im b (j r)"),
                        sb_y1[:, 0:im + 1 - lo, :, :])
                sb_ll2e = wp.tile([64, 128], F32R, tag="sbll2e")
                sb_ll2o = wp.tile([64, 128], F32R, tag="sbll2o")
                cp(sb_ll2e[:], psG2[:, 256:384])
                cp(sb_ll2o[:], psG2[:, 384:512])
                st['ll2e', im] = sb_ll2e
                st['ll2o', im] = sb_ll2o

            def emit_L3(im):
                sb_ll2e = st.pop(('ll2e', im))
                sb_ll2o = st.pop(('ll2o', im))
                psW3 = psv.tile([128, 128], F32, tag="psv")
                nc.tensor.matmul(psW3[:], sb_ll2e[:], c['RH3e'][:], start=True, stop=False)
                nc.tensor.matmul(psW3[:], sb_ll2o[:], c['RH3o'][:], start=False, stop=True)
                sb3 = wp.tile([128, 128], F32R, tag="sb3")
                cp(sb3[:], psW3[:])

                psH1 = pst.tile([32, 256], F32, tag="pst")
                nc.tensor.matmul(psH1[:], sb3[:, 0:32], c['L3g1a'][:], start=True, stop=False)
                nc.tensor.matmul(psH1[:], sb3[:, 32:64], c['L3g1b'][:], start=False, stop=True)
                psH2 = pst.tile([32, 256], F32, tag="pst")
                nc.tensor.matmul(psH2[:], sb3[:, 64:96], c['L3g2a'][:], start=True, stop=False)
                nc.tensor.matmul(psH2[:], sb3[:, 96:128], c['L3g2b'][:], start=False, stop=True)

                g2, q2 = im // G2, im % G2
                if q2 == 0:
                    sb_y2 = sp.tile([32, G2, 6, 64], F32, tag="sby2")
                    st['y2', g2] = sb_y2
                sb_y2 = st['y2', g2]
                cp(sb_y2[:, q2, 0:2, :], psH1[:, 0:128])
                cp(sb_y2[:, q2, 4:6, :], psH1[:, 128:256])
                cp(sb_y2[:, q2, 2:4, :], psH2[:, 0:128])
                if q2 == G2 - 1 or im == n_img - 1:
                    lo = g2 * G2
                    nc.sync.dma_start(
                        d_y2.ap()[lo:im + 1].rearrange("im b i j r -> i im b (j r)"),
                        sb_y2[:, 0:im + 1 - lo, :, :])

                gl, ql = im // GL, im % GL
                if ql == 0:
                    sb_l3 = sp.tile([32, GL, 128], F32, tag="sbl3")
                    st['l3', gl] = sb_l3
                sb_l3 = st['l3', gl]
                cp(sb_l3[:, ql, :], psH2[:, 128:256])
                if ql == GL - 1 or im == n_img - 1:
                    lo = gl * GL
                    nc.sync.dma_start(
                        d_ll.ap()[lo:im + 1].rearrange("im (i eo) j -> i im eo j", eo=2),
                        sb_l3[:, 0:im + 1 - lo, :].rearrange(
                            "p im (eo j) -> p im eo j", eo=2))

            # software pipeline: PE always has an adjacent independent stage;
            # lags 2/4 also delay the first use of L2/L3 constants past their DMA.
            for im in range(n_img + 4):
                if im < n_img:
                    emit_L1(im)
                if 0 <= im - 2 < n_img:
                    emit_L2(im - 2)
                if 0 <= im - 4 < n_img:
                    emit_L3(im - 4)

    nc.compile()
    return nc


_NC_CACHE = {}


def _get_nc(n_img):
    if n_img not in _NC_CACHE:
        _NC_CACHE[n_img] = build_nc(n_img)
    return _NC_CACHE[n_img]


def run_dtcwt(x_all, h0o, h1o, h0a, h0b, h1a, h1b, trace=False):
    """x_all: [n_total, 256, 256] float32, n_total = N_CORES * n_img.
    Returns (ll, yh0, yh1, yh2) stacked over n_total, plus the raw result."""
    n_total = x_all.shape[0]
    assert n_total % N_CORES == 0
    n_img = n_total // N_CORES
    nc = _get_nc(n_img)

    C = build_consts(np.asarray(h0o, np.float64), np.asarray(h1o, np.float64),
                     np.asarray(h0a, np.float64), np.asarray(h0b, np.float64),
                     np.asarray(h1a, np.float64), np.asarray(h1b, np.float64))
    cmaps = {"c_" + k: _round_f32r(C[k]) for k in CONST_SHAPES}

    xr = _round_f32r(np.asarray(x_all, np.float32))
    in_maps = []
    for i in range(N_CORES):
        m = {"x": np.ascontiguousarray(xr[i * n_img:(i + 1) * n_img])}
        m.update(cmaps)
        in_maps.append(m)

    res = run_bass_kernel_spmd(nc, in_maps, core_ids=list(range(N_CORES)),
                               trace=trace)
    ll = np.concatenate([r["ll"] for r in res.results], axis=0)
    y0 = np.concatenate([r["yh0"] for r in res.results], axis=0)
    y1 = np.concatenate([r["yh1"] for r in res.results], axis=0)
    y2 = np.concatenate([r["yh2"] for r in res.results], axis=0)
    return (ll, y0, y1, y2), res


def kernel(x, h0o, h1o, h0a, h0b, h1a, h1b):
    x = np.asarray(x, dtype=np.float32)
    B, Cc, H, W = x.shape  # (8, 16, 256, 256)
    (ll, y0, y1, y2), _ = run_dtcwt(
        x.reshape(B * Cc, H, W), h0o, h1o, h0a, h0b, h1a, h1b)
    return (ll.reshape(B, Cc, 64, 64),
            y0.reshape(B, Cc, 6, 128, 128, 2),
            y1.reshape(B, Cc, 6, 64, 64, 2),
            y2.reshape(B, Cc, 6, 32, 32, 2))


# revision 10
# speedup vs baseline: 1.0600x; 1.0600x over previous
"""DTCWT forward (3-level) Trainium2 Bass kernel.

Self-contained: builds banded-operator constant matrices from the filter
taps, shards the 8x16 (batch,channel) images across 8 NeuronCores (16
images each), and runs the whole transform as fp32r TensorEngine matmuls.

Factorization per 256x256 image (validated offline to 1e-16 vs reference):
  L1 stage1:  psV = xT-blocks @ [Fe.T|Fo.T]      (x as stationary lhsT)
  L1 stage2:  band-pair tiles  [T+|T-] = V_a@[A|A] + V_b@[B|-B]  in PSUM,
              producing q2c-combined re/im-interleaved outputs directly.
  ll path:    [ll_e|ll_o] = V3@G0llT , V4@G0llT  (row-parity split rows)
  L2/L3: same structure with dual-tree decimated operators; level inputs
  stay in row-parity-split layout, folded into the next level's constants.
"""

import numpy as np

import concourse.bacc as bacc
import concourse.mybir as mybir
from concourse import tile
from concourse.bass_utils import run_bass_kernel_spmd
from neuron_dtypes import static_cast_fp32_to_fp32r

F32 = mybir.dt.float32
F32R = mybir.dt.float32r
SQ2 = float(1.0 / np.sqrt(2.0))

N_CORES = 8
N_IMG = 16  # images per core


# ----------------------------------------------------------------------------
# Host-side constant operator matrices (float64)
# ----------------------------------------------------------------------------

def _corr(Xp, hr, Lout, stride):
    m = hr.shape[0]
    hi = stride * (Lout - 1) + 1
    acc = hr[0] * Xp[0:hi:stride]
    for k in range(1, m):
        acc = acc + hr[k] * Xp[k:k + hi:stride]
    return acc


def _colfilter_mat(h, L):
    m = h.shape[0]
    m2 = m // 2
    X = np.eye(L, dtype=np.float64)
    Xp = np.pad(X, ((m2, m2), (0, 0)), mode='symmetric')
    return _corr(Xp, h[::-1].astype(np.float64), L, 1)


def _coldfilt_mat(ha, hb, highpass, L):
    X = np.eye(L, dtype=np.float64)
    Xp = np.pad(X, ((ha.shape[0], ha.shape[0]), (0, 0)), mode='symmetric')
    E = Xp[2::2]
    O = Xp[3::2]
    a_out = _corr(O, ha[::-1].astype(np.float64), L // 4, 2)
    b_out = _corr(E, hb[::-1].astype(np.float64), L // 4, 2)
    ev, od = (b_out, a_out) if highpass else (a_out, b_out)
    out = np.zeros((L // 2, L), dtype=np.float64)
    out[0::2] = ev
    out[1::2] = od
    return out


def _pair_consts(R, n_j):
    """A/B matrices [L, 2*n_j]: T+ = Va@A + Vb@B, T- = Va@A - Vb@B give the
    q2c column combines (a-d, b+c) / (a+d, b-c) re/im-interleaved."""
    Re = R[0::2]
    Ro = R[1::2]
    L = R.shape[1]
    A = np.zeros((L, 2 * n_j))
    B = np.zeros((L, 2 * n_j))
    A[:, 0::2] = Re.T
    A[:, 1::2] = Ro.T
    B[:, 0::2] = -Ro.T
    B[:, 1::2] = Re.T
    return A, B


def build_consts(h0o, h1o, h0a, h0b, h1a, h1b):
    C = {}
    # L1 (undecimated, 256)
    F0 = _colfilter_mat(h0o, 256)
    F1 = _colfilter_mat(h1o, 256)
    F1e, F1o = SQ2 * F1[0::2], SQ2 * F1[1::2]
    F0e, F0o = SQ2 * F0[0::2], SQ2 * F0[1::2]
    C['RH01'] = np.concatenate([F1e.T, F1o.T, F0e.T, F0o.T], axis=1)  # [256,512]
    A1, B1 = _pair_consts(F0, 128)
    A2, B2 = _pair_consts(F1, 128)
    C['P1a'] = np.concatenate([A1, A1], axis=1)                 # [256,512]
    C['P1b'] = np.concatenate([B1, -B1], axis=1)
    C['P2a'] = np.concatenate([A2, A2], axis=1)
    C['P2b'] = np.concatenate([B2, -B2], axis=1)
    C['G0ll'] = (1.0 / SQ2) * F0.T                              # [256,256]
    # L2 (256 -> 128)
    C0 = _coldfilt_mat(h0b, h0a, False, 256)
    C1 = _coldfilt_mat(h1b, h1a, True, 256)
    C1e, C1o = SQ2 * C1[0::2], SQ2 * C1[1::2]
    C0e, C0o = SQ2 * C0[0::2], SQ2 * C0[1::2]
    RH2 = np.concatenate([C1e.T, C1o.T, C0e.T, C0o.T], axis=1)  # [256,256]
    C['RH2e'] = RH2[0::2]                                       # [128,256]
    C['RH2o'] = RH2[1::2]
    A1_, B1_ = _pair_consts(C0, 64)
    A2_, B2_ = _pair_consts(C1, 64)
    C['L2g1a'] = np.concatenate([A1_, A2_, A2_, A1_], axis=1)   # [256,512]
    C['L2g1b'] = np.concatenate([B1_, B2_, -B2_, -B1_], axis=1)
    Z = np.zeros((256, 128))
    C['L2g2a'] = np.concatenate([A2_, A2_, (1 / SQ2) * C0.T, Z], axis=1)
    C['L2g2b'] = np.concatenate([B2_, -B2_, Z, (1 / SQ2) * C0.T], axis=1)
    # L3 (128 -> 64)
    C0_ = _coldfilt_mat(h0b, h0a, False, 128)
    C1_ = _coldfilt_mat(h1b, h1a, True, 128)
    C1pe, C1po = SQ2 * C1_[0::2], SQ2 * C1_[1::2]
    C0pe, C0po = SQ2 * C0_[0::2], SQ2 * C0_[1::2]
    RH3 = np.concatenate([C1pe.T, C1po.T, C0pe.T, C0po.T], axis=1)  # [128,128]
    C['RH3e'] = RH3[0::2]                                       # [64,128]
    C['RH3o'] = RH3[1::2]
    A1__, B1__ = _pair_consts(C0_, 32)
    A2__, B2__ = _pair_consts(C1_, 32)
    C['L3g1a'] = np.concatenate([A1__, A2__, A2__, A1__], axis=1)   # [128,256]
    C['L3g1b'] = np.concatenate([B1__, B2__, -B2__, -B1__], axis=1)
    Z3 = np.zeros((128, 64))
    C['L3g2a'] = np.concatenate([A2__, A2__, (1 / SQ2) * C0_.T, Z3], axis=1)
    C['L3g2b'] = np.concatenate([B2__, -B2__, Z3, (1 / SQ2) * C0_.T], axis=1)
    return C


CONST_SHAPES = {
    'RH01': (256, 512),
    'P1a': (256, 512), 'P1b': (256, 512), 'P2a': (256, 512), 'P2b': (256, 512),
    'G0ll': (256, 256),
    'RH2e': (128, 256), 'RH2o': (128, 256),
    'L2g1a': (256, 512), 'L2g1b': (256, 512),
    'L2g2a': (256, 512), 'L2g2b': (256, 512),
    'RH3e': (64, 128), 'RH3o': (64, 128),
    'L3g1a': (128, 256), 'L3g1b': (128, 256),
    'L3g2a': (128, 256), 'L3g2b': (128, 256),
}


def _round_f32r(a):
    a = np.ascontiguousarray(a.astype(np.float32))
    return static_cast_fp32_to_fp32r(a).view(np.float32).reshape(a.shape)


# ----------------------------------------------------------------------------
# Device program
# ----------------------------------------------------------------------------

def build_nc(n_img=N_IMG):
    nc = bacc.Bacc("TRN2", target_bir_lowering=False, debug=False)

    x = nc.dram_tensor("x", [n_img, 256, 256], F32R, kind="ExternalInput")
    d_ll = nc.dram_tensor("ll", [n_img, 64, 64], F32, kind="ExternalOutput")
    d_y0 = nc.dram_tensor("yh0", [n_img, 6, 128, 128, 2], F32, kind="ExternalOutput")
    d_y1 = nc.dram_tensor("yh1", [n_img, 6, 64, 64, 2], F32, kind="ExternalOutput")
    d_y2 = nc.dram_tensor("yh2", [n_img, 6, 32, 32, 2], F32, kind="ExternalOutput")

    d_c = {k: nc.dram_tensor("c_" + k, list(v), F32R, kind="ExternalInput")
           for k, v in CONST_SHAPES.items()}

    # output-staging group sizes (images per DMA)
    G0 = min(2, n_img)
    G1 = min(2, n_img)
    G2 = min(4, n_img)
    GL = min(8, n_img)
    GX = min(2, n_img)

    with tile.TileContext(nc) as tc:
        with tc.tile_pool(name="consts", bufs=1) as cpool, \
             tc.tile_pool(name="work", bufs=3) as wp, \
             tc.tile_pool(name="stage", bufs=2) as sp, \
             tc.tile_pool(name="psv", bufs=3, space="PSUM") as psv, \
             tc.tile_pool(name="pst", bufs=5, space="PSUM") as pst:

            st = {}           # per-image / per-group live tiles
            cp_i = [0]

            def cp(dst, src):
                # alternate copies between ScalarE and VectorE
                if cp_i[0] % 2 == 0:
                    nc.scalar.copy(dst, src)
                else:
                    nc.vector.tensor_copy(dst, src)
                cp_i[0] += 1

            # ---- first x-load leads the SP ring so image 0 starts immediately
            sbx0 = wp.tile([128, GX, 2, 256], F32R, tag="sbx")
            nc.sync.dma_start(sbx0[:], x.ap()[0:GX].rearrange(
                "im (hc p) w -> p im hc w", p=128))
            st['x', 0] = sbx0

            # ---- L1 constants on the ACT ring (use order); L2/L3 on SP after x
            L1_CONSTS = ('RH01', 'P1a', 'P1b', 'P2a', 'P2b', 'G0ll')
            c = {}
            for k, (K, N) in CONST_SHAPES.items():
                eng = nc.scalar if k in L1_CONSTS else nc.sync
                if K > 128:
                    t = cpool.tile([128, K // 128, N], F32R, tag="c_" + k)
                    eng.dma_start(
                        t[:], d_c[k].ap().rearrange("(kc p) n -> p kc n", p=128))
                else:
                    t = cpool.tile([K, N], F32R, tag="c_" + k)
                    eng.dma_start(t[:], d_c[k].ap())
                c[k] = t

            def emit_L1(im):
                g, q = im // GX, im % GX
                if q == 0 and ('x', g) not in st:
                    sbx = wp.tile([128, GX, 2, 256], F32R, tag="sbx")
                    nc.sync.dma_start(
                        sbx[:], x.ap()[g * GX:(g + 1) * GX].rearrange(
                            "im (hc p) w -> p im hc w", p=128))
                    st['x', g] = sbx
                sbx = st['x', g]
                psVa = psv.tile([128, 512], F32, tag="psv")
                psVb = psv.tile([128, 512], F32, tag="psv")
                for wb, psV in ((0, psVa), (1, psVb)):
                    for hc in range(2):
                        nc.tensor.matmul(psV[:], sbx[:, q, hc, wb * 128:(wb + 1) * 128],
                                         c['RH01'][:, hc, :],
                                         start=(hc == 0), stop=(hc == 1))
                sbV = wp.tile([128, 2, 512], F32R, tag="sbv")
                cp(sbV[:, 0, :], psVa[:])
                cp(sbV[:, 1, :], psVb[:])

                g0, q0 = im // G0, im % G0
                if q0 == 0:
                    sb_y0 = sp.tile([128, G0, 6, 256], F32, tag="sby0")
                    st['y0', g0] = sb_y0
                sb_y0 = st['y0', g0]
                # band pairs (bands b0,b1 per pair): psT = [T+ | T-]
                for base, ka, kb, b0, b1 in ((0, 'P1a', 'P1b', 0, 5),
                                             (0, 'P2a', 'P2b', 1, 4),
                                             (256, 'P2a', 'P2b', 2, 3)):
                    psT = pst.tile([128, 512], F32, tag="pst")
                    for c2 in range(2):
                        nc.tensor.matmul(psT[:], sbV[:, c2, base:base + 128],
                                         c[ka][:, c2, :], start=(c2 == 0), stop=False)
                        nc.tensor.matmul(psT[:], sbV[:, c2, base + 128:base + 256],
                                         c[kb][:, c2, :], start=False, stop=(c2 == 1))
                    cp(sb_y0[:, q0, b0, :], psT[:, 0:256])
                    cp(sb_y0[:, q0, b1, :], psT[:, 256:512])
                if q0 == G0 - 1 or im == n_img - 1:
                    lo = g0 * G0
                    nc.sync.dma_start(
                        d_y0.ap()[lo:im + 1].rearrange("im b i j r -> i im b (j r)"),
                        sb_y0[:, 0:im + 1 - lo, :, :])

                # ll pack [T_lle | T_llo]
                ps_ll = pst.tile([128, 512], F32, tag="pst")
                for c2 in range(2):
                    nc.tensor.matmul(ps_ll[:, 0:256], sbV[:, c2, 256:384],
                                     c['G0ll'][:, c2, :], start=(c2 == 0), stop=(c2 == 1))
                for c2 in range(2):
                    nc.tensor.matmul(ps_ll[:, 256:512], sbV[:, c2, 384:512],
                                     c['G0ll'][:, c2, :], start=(c2 == 0), stop=(c2 == 1))
                sb_ll = wp.tile([128, 512], F32R, tag="sbll")
                cp(sb_ll[:], ps_ll[:])
                st['ll', im] = sb_ll

            def emit_L2(im):
                sb_ll = st.pop(('ll', im))
                psW = psv.tile([128, 512], F32, tag="psv")
                for wb in range(2):
                    for eo in range(2):
                        lhsT = sb_ll[:, eo * 256 + wb * 128: eo * 256 + (wb + 1) * 128]
                        rhs = c['RH2e'] if eo == 0 else c['RH2o']
                        nc.tensor.matmul(psW[:, wb * 256:(wb + 1) * 256],
                                         lhsT, rhs[:], start=(eo == 0), stop=(eo == 1))
                sbW = wp.tile([128, 512], F32R, tag="sbw")
                cp(sbW[:], psW[:])

                psG1 = pst.tile([64, 512], F32, tag="pst")
                for c2 in range(2):
                    nc.tensor.matmul(psG1[:], sbW[:, c2 * 256 + 0:c2 * 256 + 64],
                                     c['L2g1a'][:, c2, :], start=(c2 == 0), stop=False)
                    nc.tensor.matmul(psG1[:], sbW[:, c2 * 256 + 64:c2 * 256 + 128],
                                     c['L2g1b'][:, c2, :], start=False, stop=(c2 == 1))
                psG2 = pst.tile([64, 512], F32, tag="pst")
                for c2 in range(2):
                    nc.tensor.matmul(psG2[:], sbW[:, c2 * 256 + 128:c2 * 256 + 192],
                                     c['L2g2a'][:, c2, :], start=(c2 == 0), stop=False)
                    nc.tensor.matmul(psG2[:], sbW[:, c2 * 256 + 192:c2 * 256 + 256],
                                     c['L2g2b'][:, c2, :], start=False, stop=(c2 == 1))

                g1, q1 = im // G1, im % G1
                if q1 == 0:
                    sb_y1 = sp.tile([64, G1, 6, 128], F32, tag="sby1")
                    st['y1', g1] = sb_y1
                sb_y1 = st['y1', g1]
                cp(sb_y1[:, q1, 0:2, :], psG1[:, 0:256])
                cp(sb_y1[:, q1, 4:6, :], psG1[:, 256:512])
                cp(sb_y1[:, q1, 2:4, :], psG2[:, 0:256])
                if q1 == G1 - 1 or im == n_img - 1:
                    lo = g1 * G1
                    nc.sync.dma_start(
                        d_y1.ap()[lo:im + 1].rearrange("im b i j r -> i im b (j r)"),
                        sb_y1[:, 0:im + 1 - lo, :, :])
                sb_ll2e = wp.tile([64, 128], F32R, tag="sbll2e")
                sb_ll2o = wp.tile([64, 128], F32R, tag="sbll2o")
                cp(sb_ll2e[:], psG2[:, 256:384])
                cp(sb_ll2o[:], psG2[:, 384:512])
                st['ll2e', im] = sb_ll2e
                st['ll2o', im] = sb_ll2o

            def emit_L3(im):
                sb_ll2e = st.pop(('ll2e', im))
                sb_ll2o = st.pop(('ll2o', im))
                psW3 = psv.tile([128, 128], F32, tag="psv")
                nc.tensor.matmul(psW3[:], sb_ll2e[:], c['RH3e'][:], start=True, stop=False)
                nc.tensor.matmul(psW3[:], sb_ll2o[:], c['RH3o'][:], start=False, stop=True)
                sb3 = wp.tile([128, 128], F32R, tag="sb3")
                cp(sb3[:], psW3[:])

                psH1 = pst.tile([32, 256], F32, tag="pst")
                nc.tensor.matmul(psH1[:], sb3[:, 0:32], c['L3g1a'][:], start=True, stop=False)
                nc.tensor.matmul(psH1[:], sb3[:, 32:64], c['L3g1b'][:], start=False, stop=True)
                psH2 = pst.tile([32, 256], F32, tag="pst")
                nc.tensor.matmul(psH2[:], sb3[:, 64:96], c['L3g2a'][:], start=True, stop=False)
                nc.tensor.matmul(psH2[:], sb3[:, 96:128], c['L3g2b'][:], start=False, stop=True)

                g2, q2 = im // G2, im % G2
                if q2 == 0:
                    sb_y2 = sp.tile([32, G2, 6, 64], F32, tag="sby2")
                    st['y2', g2] = sb_y2
                sb_y2 = st['y2', g2]
                cp(sb_y2[:, q2, 0:2, :], psH1[:, 0:128])
                cp(sb_y2[:, q2, 4:6, :], psH1[:, 128:256])
                cp(sb_y2[:, q2, 2:4, :], psH2[:, 0:128])
                if q2 == G2 - 1 or im == n_img - 1:
                    lo = g2 * G2
                    nc.sync.dma_start(
                        d_y2.ap()[lo:im + 1].rearrange("im b i j r -> i im b (j r)"),
                        sb_y2[:, 0:im + 1 - lo, :, :])

                gl, ql = im // GL, im % GL
                if ql == 0:
                    sb_l3 = sp.tile([32, GL, 128], F32, tag="sbl3")
                    st['l3', gl] = sb_l3
                sb_l3 = st['l3', gl]
                cp(sb_l3[:, ql, :], psH2[:, 128:256])
                if ql == GL - 1 or im == n_img - 1:
                    lo = gl * GL
                    nc.sync.dma_start(
                        d_ll.ap()[lo:im + 1].rearrange("im (i eo) j -> i im eo j", eo=2),
                        sb_l3[:, 0:im + 1 - lo, :].rearrange(
                            "p im (eo j) -> p im eo j", eo=2))

            # software pipeline: PE always has an adjacent independent stage;
            # lags 2/4 also delay the first use of L2/L3 constants past their DMA.
            for im in range(n_img + 4):
                if im < n_img:
                    emit_L1(im)
                if 0 <= im - 2 < n_img:
                    emit_L2(im - 2)
                if 0 <= im - 4 < n_img:
                    emit_L3(im - 4)

    nc.compile()
    return nc


_NC_CACHE = {}


def _get_nc(n_img):
    if n_img not in _NC_CACHE:
        _NC_CACHE[n_img] = build_nc(n_img)
    return _NC_CACHE[n_img]


def run_dtcwt(x_all, h0o, h1o, h0a, h0b, h1a, h1b, trace=False):
    """x_all: [n_total, 256, 256] float32, n_total = N_CORES * n_img.
    Returns (ll, yh0, yh1, yh2) stacked over n_total, plus the raw result."""
    n_total = x_all.shape[0]
    assert n_total % N_CORES == 0
    n_img = n_total // N_CORES
    nc = _get_nc(n_img)

    C = build_consts(np.asarray(h0o, np.float64), np.asarray(h1o, np.float64),
                     np.asarray(h0a, np.float64), np.asarray(h0b, np.float64),
                     np.asarray(h1a, np.float64), np.asarray(h1b, np.float64))
    cmaps = {"c_" + k: _round_f32r(C[k]) for k in CONST_SHAPES}

    xr = _round_f32r(np.asarray(x_all, np.float32))
    in_maps = []
    for i in range(N_CORES):
        m = {"x": np.ascontiguousarray(xr[i * n_img:(i + 1) * n_img])}
        m.update(cmaps)
        in_maps.append(m)

    res = run_bass_kernel_spmd(nc, in_maps, core_ids=list(range(N_CORES)),
                               trace=trace)
    ll = np.concatenate([r["ll"] for r in res.results], axis=0)
    y0 = np.concatenate([r["yh0"] for r in res.results], axis=0)
    y1 = np.concatenate([r["yh1"] for r in res.results], axis=0)
    y2 = np.concatenate([r["yh2"] for r in res.results], axis=0)
    return (ll, y0, y1, y2), res


def kernel(x, h0o, h1o, h0a, h0b, h1a, h1b):
    x = np.asarray(x, dtype=np.float32)
    B, Cc, H, W = x.shape  # (8, 16, 256, 256)
    (ll, y0, y1, y2), _ = run_dtcwt(
        x.reshape(B * Cc, H, W), h0o, h1o, h0a, h0b, h1a, h1b)
    return (ll.reshape(B, Cc, 64, 64),
            y0.reshape(B, Cc, 6, 128, 128, 2),
            y1.reshape(B, Cc, 6, 64, 64, 2),
            y2.reshape(B, Cc, 6, 32, 32, 2))
